# revision 4
# baseline (speedup 1.0000x reference)
"""DAGPool kernel for Trainium2 (8 NeuronCores, SPMD data-parallel over units).

Problem structure (hardcoded, validated at runtime against the edges input):
  - 2048 independent units of 128 nodes: node 0 is a root that bifurcates
    into two chains (nodes 1..63 and nodes 64..127).
  - The DAGPool traversal with cluster_size=4 yields, per unit, 33 clusters
    over *contiguous* node ranges:
      cluster u                      <- node 0            (count 1)
      2048 + u*16 + j   (j=0..15)    <- nodes 1+4j..       (count 4, last 3)
      2048+32768 + u*16 + j (j=0..15)<- nodes 64+4j..+3    (count 4)
  - new_nodes = segment mean (memory-bound strided pooling, done on device)
  - new_edge_index = sorted unique remapped edges (int-only, done on host)

Device strategy: shard 2048 units across 8 cores (256 units each). Per core,
two macro-tiles of 128 units with SBUF partition = unit, free = (row, D).
Input DMAs are 32KB-contiguous per partition; group sums are innermost-axis
reduce_sum on the vector engine; scale on the scalar engine; stores are
8KB-contiguous.
"""

import numpy as np

UNITS = 2048
UNIT = 128
N = UNITS * UNIT
D = 128
CLUS_MAX = 4
CC = UNITS * 33  # 67584 clusters total

N_CORES = 8
UPC = UNITS // N_CORES          # units per core: 256
ROWS_PC = UPC * UNIT            # input rows per core: 32768
OUT_PC = UPC * 33               # output rows per core: 8448

TRACE = False                   # test.py sets True to collect an NTFF profile
LAST_RESULTS = None             # BassKernelResults of the last device run


def _expected_edges():
    bases = np.arange(UNITS, dtype=np.int64) * UNIT
    c1s = (bases[:, None] + np.arange(63, dtype=np.int64)).ravel()
    c1d = c1s + 1
    c2s = (bases[:, None] + 64 + np.arange(63, dtype=np.int64)).ravel()
    c2d = c2s + 1
    src = np.concatenate([c1s, c2s, bases])
    dst = np.concatenate([c1d, c2d, bases + 64])
    return np.stack([src, dst])


def _clusters_closed_form():
    cl = np.empty((UNITS, UNIT), dtype=np.int64)
    u = np.arange(UNITS, dtype=np.int64)[:, None]
    cl[:, 0:1] = u
    k1 = np.arange(63, dtype=np.int64)
    cl[:, 1:64] = UNITS + u * 16 + (k1 // 4)
    k2 = np.arange(64, dtype=np.int64)
    cl[:, 64:128] = UNITS + UNITS * 16 + u * 16 + (k2 // 4)
    return cl.ravel()


def _new_edge_index(cluster, edges_np, cc):
    ce0 = cluster[edges_np[0].astype(np.int64)]
    ce1 = cluster[edges_np[1].astype(np.int64)]
    lin = np.unique(ce0 * np.int64(cc) + ce1)
    return np.stack([lin // cc, lin % cc]).astype(np.int32)


_NC = None


def _split_multi_waits(nc):
    """The walrus build in this container accepts at most one sync-wait per
    instruction (two for EventSemaphore); Tile emits more. Hoist excess waits
    onto same-engine InstNoOp carriers inserted right before the instruction
    (the engine queue executes them in order, so the semantics are
    unchanged)."""
    import bass_rust
    import concourse.mybir as mybir

    fn = nc.m.functions[0]
    for blk in fn.blocks:
        new_list = []
        changed = False
        for inst in blk.instructions:
            si = inst.sync_info
            cap = 2 if isinstance(inst, mybir.InstEventSemaphore) else 1
            if si is not None and len(si.on_wait) > cap:
                waits = list(si.on_wait)
                extra, keep = waits[:-cap], waits[-cap:]
                for k, w in enumerate(extra):
                    nop = bass_rust.InstNoOp(name=f"{inst.name}-ws{k}")
                    nop.engine = inst.engine
                    nop.sync_info = bass_rust.SyncInfo(on_wait=[w], on_update=[])
                    new_list.append(nop)
                inst.sync_info = bass_rust.SyncInfo(
                    on_wait=keep, on_update=list(si.on_update))
                changed = True
            new_list.append(inst)
        if changed:
            blk.instructions = new_list
    return nc


def _build_bass():
    import concourse.bass as bass
    import concourse.mybir as mybir
    from concourse.tile import TileContext

    nc = bass.Bass()
    x = nc.dram_tensor("x", [ROWS_PC, D], mybir.dt.float32, kind="ExternalInput")
    y = nc.dram_tensor("y", [OUT_PC, D], mybir.dt.float32, kind="ExternalOutput")
    xv = x[:].rearrange("(u r) d -> u r d", r=UNIT)      # [256, 128, 128]
    yr = y[0:UPC, :]                                     # roots block
    y1 = y[UPC:UPC * 17, :].rearrange("(u j) d -> u j d", j=16)   # chain1 block
    y2 = y[UPC * 17:UPC * 33, :].rearrange("(u j) d -> u j d", j=16)  # chain2 block

    with TileContext(nc) as tc:
        with tc.tile_pool(name="io", bufs=2) as pool:
            for m in range(UPC // 128):                  # 2 macro-tiles of 128 units
                U0 = m * 128
                in_lo = pool.tile([128, 64, D], mybir.dt.float32)
                in_hi = pool.tile([128, 64, D], mybir.dt.float32)
                nc.sync.dma_start(in_lo[:], xv[U0:U0 + 128, 0:64, :])
                nc.sync.dma_start(in_hi[:], xv[U0:U0 + 128, 64:128, :])
                # roots: plain copy of row 0 of each unit
                nc.sync.dma_start(yr[U0:U0 + 128, :], in_lo[:, 0, :])
                s1 = pool.tile([128, 16, D], mybir.dt.float32)
                s2 = pool.tile([128, 16, D], mybir.dt.float32)
                out1 = pool.tile([128, 16, D], mybir.dt.float32)
                out2 = pool.tile([128, 16, D], mybir.dt.float32)
                # chain1 groups j=0..14: rows 1..60, groups of 4
                nc.vector.reduce_sum(
                    s1[:, 0:15, :],
                    in_lo[:, 1:61, :].rearrange("p (j t) d -> p j d t", t=4),
                    axis=mybir.AxisListType.X)
                # chain1 group 15: rows 61..63 (3 nodes)
                nc.vector.reduce_sum(
                    s1[:, 15, :],
                    in_lo[:, 61:64, :].rearrange("p t d -> p d t"),
                    axis=mybir.AxisListType.X)
                # chain2: rows 64..127, 16 groups of 4
                nc.vector.reduce_sum(
                    s2[:],
                    in_hi[:].rearrange("p (j t) d -> p j d t", t=4),
                    axis=mybir.AxisListType.X)
                # group 15 has 3 members: pre-scale by 4/3 so one uniform
                # 0.25 scale finishes the mean (single producer per out tile,
                # keeping each store DMA to one sync wait)
                nc.vector.tensor_scalar_mul(s1[:, 15, :], s1[:, 15, :], 4.0 / 3.0)
                nc.scalar.mul(out1[:], s1[:], 0.25)
                nc.scalar.mul(out2[:], s2[:], 0.25)
                nc.sync.dma_start(y1[U0:U0 + 128, :, :], out1[:])
                nc.sync.dma_start(y2[U0:U0 + 128, :, :], out2[:])
    return _split_multi_waits(nc)


def _run_device(x_np):
    global _NC, LAST_RESULTS
    from concourse import bass_utils

    if _NC is None:
        _NC = _build_bass()
    in_maps = [{"x": np.ascontiguousarray(x_np[c * ROWS_PC:(c + 1) * ROWS_PC])}
               for c in range(N_CORES)]
    res = bass_utils.run_bass_kernel_spmd(
        _NC, in_maps, core_ids=list(range(N_CORES)), trace=TRACE)
    LAST_RESULTS = res
    ys = [res.results[c]["y"] for c in range(N_CORES)]
    roots = np.concatenate([y[0:UPC] for y in ys])
    chain1 = np.concatenate([y[UPC:UPC * 17] for y in ys])
    chain2 = np.concatenate([y[UPC * 17:UPC * 33] for y in ys])
    return np.concatenate([roots, chain1, chain2], axis=0)


# ---------------------------------------------------------------------------
# Generic host fallback (only taken if the edges input ever deviates from the
# fixed forest built by the reference's build_edges()).

def _compute_clusters_generic(edges_np, n, clus_max):
    src, dst = edges_np[0].astype(np.int64), edges_np[1].astype(np.int64)
    keep = src != dst
    src, dst = src[keep], dst[keep]
    outdeg = np.bincount(src, minlength=n)
    bif = np.nonzero(outdeg > 1)[0]
    bif_set = set(bif.tolist())
    roots = sorted(set(range(n)) - set(dst.tolist()))
    roots = roots + dst[np.isin(src, bif)].tolist()
    nxt = np.full(n, -1, dtype=np.int64)
    for s, d_ in zip(src.tolist(), dst.tolist()):
        if s not in bif_set:
            nxt[s] = d_
    cluster = np.zeros(n, dtype=np.int64)
    cc = 0
    for r in roots:
        i = 0
        node = r
        while node >= 0:
            if i >= clus_max:
                cc += 1
                i = 0
            cluster[node] = cc
            i += 1
            node = nxt[node]
        cc += 1
    return cluster, cc


def _fallback(x_np, edges_np):
    cluster, cc = _compute_clusters_generic(edges_np, x_np.shape[0], CLUS_MAX)
    sums = np.zeros((cc, x_np.shape[1]), dtype=np.float64)
    np.add.at(sums, cluster, x_np.astype(np.float64))
    counts = np.bincount(cluster, minlength=cc).astype(np.float64)
    new_nodes = (sums / np.maximum(counts, 1.0)[:, None]).astype(np.float32)
    return new_nodes, _new_edge_index(cluster, edges_np, cc)


def kernel(x, edges):
    x_np = np.asarray(x, dtype=np.float32)
    edges_in = edges
    edges_np = np.asarray(edges)

    if (x_np.shape == (N, D)
            and np.array_equal(edges_np.astype(np.int64), _expected_edges())):
        cluster = _clusters_closed_form()
        new_nodes = _run_device(x_np)
        new_edge_index = _new_edge_index(cluster, edges_np, CC)
    else:
        new_nodes, new_edge_index = _fallback(x_np, edges_np)

    return new_nodes, new_edge_index, x, edges_in
